# revision 72
# baseline (speedup 1.0000x reference)
"""MQA attention (B=2, Lq=Lkv=2048, F=1024, H=16, D=64) on 8 TRN2 cores.

Sharding: core = (batch, query-block-of-512). Each core computes its full
output rows (all 16 heads + output projection) -> no collectives; host
concatenates per-core yT slabs.

v2 dataflow (bf16/fp16 matmul operands, f32 PSUM accumulation):
  KV chain first (per 512-row l-block): kvT = Wkv.T @ xkvT -> RoPE-k
  (halves-permuted basis, swap via small PE matmul) -> ktop/kbot bf16;
  V transposed into vaug fp16 (ones col 64 = softmax denominator row).
  Q proj per head-pair j (interleaved into the attention pair loop):
  qT = Wq_j.T @ xqT -> RoPE -> qrot bf16.
  Attention per (pair j, kv-chunk c): S^T x2 (ktop/kbot stationary) ->
  exp on ACT ([128,2,512] PSUM supertile -> fp16) -> mask mul x2 on DVE
  (fp16 2x mode) -> O accumulation x2 (vaug stationary).
  Pair epilogue off the PE critical path: denominator rows -> DVE
  reciprocal_approx_fast -> fp16 -> K=1 ones matmul broadcast -> DVE
  normalize muls -> obig bf16 (head 2j+1 half moved down via gpsimd DMA).
  yT = Wo-chunks.T @ obig (+bo), Wo fully prefetched during attention.
"""

import ml_dtypes
import numpy as np

import concourse.bass as bass
import concourse.tile as tile
from concourse import bacc, mybir
from concourse import bass_utils
from concourse.bass import ts
from concourse.masks import make_identity

F32 = mybir.dt.float32
BF16 = mybir.dt.bfloat16
FP16 = mybir.dt.float16

B, L, F, H, D = 2, 2048, 1024, 16, 64
LQ = 512            # query rows per core
LK = 2048           # kv rows (full)
NCORES = 8
PAIRS = H // 2      # head pairs (one qT partition block each)
FCH = F // 128      # f contraction chunks
KCH = LK // 128     # lk chunks
NL = LK // LQ       # kv l-blocks

_CACHED = {}


def build_nc():
    nc = bacc.Bacc("TRN2", target_bir_lowering=False, debug=False,
                   num_devices=NCORES)
    dt_in = [
        ("xq_t", [128, FCH, LQ], BF16),        # [p, f, lq] - 8KB DMA lines
        ("xkv_t", [NL, 128, FCH, LQ], BF16),   # [l, p, f, lq]
        ("mask_t", [128, KCH, 2, LQ], FP16),   # [p, c, tt, lq] pre-duplicated
        ("wq", [FCH, 128, FCH, 128], BF16),    # [j, p, f, m]
        ("wkv", [128, FCH, 128], BF16),        # [p, f, m]
        ("wo", [FCH, 128, FCH, 128], BF16),    # [fb, p, j, m]
        ("bqbo", [128, 2 * FCH], F32),         # cols 0:8 bq-blocks, 8:16 bo
        ("bkv", [2 * D], F32),
        ("cosq", [128, LQ], BF16),
        ("sinq", [128, LQ], BF16),
        ("cksk", [D, 2 * LK], BF16),           # [p, (cos|sin)*lk]
    ]
    t = {name: nc.dram_tensor(name, shape, dt, kind="ExternalInput")
         for name, shape, dt in dt_in}
    yT = nc.dram_tensor("yT", [F, LQ], F32, kind="ExternalOutput")

    with tile.TileContext(nc) as tc:
        with (
            tc.tile_pool(name="persist", bufs=1) as persist,
            tc.tile_pool(name="ptiles", bufs=3) as ptp,
            tc.tile_pool(name="small", bufs=4) as small,
            tc.tile_pool(name="xin", bufs=2) as xin,
            tc.tile_pool(name="wst", bufs=2) as wst,
            tc.tile_pool(name="qraw", bufs=2) as qrp,
            tc.tile_pool(name="kvraw", bufs=2) as kvp,
            tc.tile_pool(name="ropetmp", bufs=2) as rtp,
            tc.tile_pool(name="rec", bufs=2) as recp,
            tc.tile_pool(name="yout", bufs=2) as yout,
            tc.tile_pool(name="psa", bufs=2, space="PSUM") as psa,   # 2 banks
            tc.tile_pool(name="psb", bufs=2, space="PSUM") as psb,   # 2 banks
            tc.tile_pool(name="psst", bufs=2, space="PSUM") as psst,  # 4 banks
        ):
            # ---------------- small constants (gpsimd DMA queue) ---------
            cq = persist.tile([128, LQ], BF16)
            sq = persist.tile([128, LQ], BF16)
            cksk = persist.tile([D, 2, LK], BF16)
            # tables lead the scalar queue (ACT is idle in the lead-in;
            # each engine queue has its own DMA descriptor pipe, so
            # spreading input DMAs across queues parallelizes transfers)
            nc.scalar.dma_start(cq, t["cosq"].ap())
            nc.scalar.dma_start(sq, t["sinq"].ap())
            nc.scalar.dma_start(cksk,
                                t["cksk"].ap().rearrange("p (a l) -> p a l",
                                                         a=2))
            ck = cksk[:, 0, :]
            sk = cksk[:, 1, :]
            bqbo = small.tile([128, 2 * FCH], F32, tag="bias")
            nc.scalar.dma_start(bqbo, t["bqbo"].ap())
            bq_sb = bqbo[:, 0:FCH]
            bo_sb = bqbo[:, FCH:2 * FCH]
            bkv_sb = small.tile([128, 1], F32, tag="bias2")
            nc.scalar.dma_start(bkv_sb, t["bkv"].ap().unsqueeze(1))

            # mask chunks, host-duplicated per head-half: the pt multiply
            # is a single contiguous free-size-1024 DVE op AND the DMA
            # moves 8KB-contiguous per-partition lines.
            mt2 = persist.tile([128, KCH, 2, LQ], FP16)

            # ---------------- persistent compute tiles -------------------
            qrot = persist.tile([128, PAIRS, LQ], BF16)
            ktop = persist.tile([128, LK], BF16)          # k rows 0:64
            kbot = persist.tile([128, LK], BF16)          # k rows 64:128
            vaug = persist.tile([128, KCH, D + 1], FP16)  # V chunks + ones col
            obig = persist.tile([128, PAIRS, LQ], BF16)   # normalized O^T

            idt = small.tile([128, 128], F32, tag="ident")
            make_identity(nc, idt)
            idtb = small.tile([128, 128], BF16, tag="identb")
            nc.vector.tensor_copy(idtb, idt)
            # halves-swap permutation matrix: M[p, p-xor-32-within-head] = 1
            swpf = small.tile([128, 128], F32, tag="swpf")
            nc.gpsimd.memset(swpf, 0.0)
            for o1, o2 in ((0, 32), (32, 0), (64, 96), (96, 64)):
                nc.gpsimd.affine_select(
                    out=swpf[o1:o1 + 32, o2:o2 + 32],
                    in_=swpf[o1:o1 + 32, o2:o2 + 32],
                    compare_op=mybir.AluOpType.not_equal, fill=1.0,
                    base=0, pattern=[[-1, 32]], channel_multiplier=1)
            swp = small.tile([128, 128], BF16, tag="swp")
            nc.vector.tensor_copy(swp, swpf)
            nc.vector.memset(ktop[64:128], 0.0)
            nc.vector.memset(kbot[0:64], 0.0)
            nc.vector.memset(vaug[:, :, D:D + 1], 1.0)

            # ======== phase A/B: KV chain + Q projections, interleaved ====
            # Sync-queue DMA order IS the bandwidth priority order: wkv,
            # xkv_l0, xq, wq0, xkv_l1, wq1, ... One batched DMA per block
            # (DMA issue on the queue engine costs ~600ns per instruction).
            wkv_sb = wst.tile([128, FCH, 128], BF16, tag="wkv")
            nc.sync.dma_start(wkv_sb, t["wkv"].ap())
            xq = persist.tile([128, FCH, LQ], BF16)
            xkvs = []
            wqs = []
            # xkv in halves for finer matmul deps, round-robined over
            # queues so the per-queue descriptor pipes run in parallel
            kvq = [nc.sync, nc.gpsimd, nc.sync, nc.gpsimd,
                   nc.sync, nc.gpsimd, nc.sync, nc.gpsimd]
            for l in range(NL):
                xkv = xin.tile([128, FCH, LQ], BF16, tag="x", bufs=NL)
                for h in range(2):
                    kvq[2 * l + h].dma_start(
                        xkv[:, 4 * h:4 * h + 4, :],
                        t["xkv_t"].ap()[l][:, 4 * h:4 * h + 4, :])
                xkvs.append(xkv)
                if l == 0:
                    nc.scalar.dma_start(xq, t["xq_t"].ap())
                    wq_0 = wst.tile([128, FCH, 128], BF16, tag="wq", bufs=8)
                    nc.scalar.dma_start(wq_0, t["wq"].ap()[0])
                    wqs.append(wq_0)
            # mask stream in quarter granularity so pair-0 chunk c only
            # waits on its own quarter
            mq_eng = [nc.scalar, nc.gpsimd, nc.sync, nc.scalar]
            for mq in range(4):
                mq_eng[mq].dma_start(
                    mt2[:, 4 * mq:4 * mq + 4, :, :],
                    t["mask_t"].ap()[:, 4 * mq:4 * mq + 4, :, :])
            for j in range(1, PAIRS):
                wq_j = wst.tile([128, FCH, 128], BF16, tag="wq", bufs=8)
                (nc.sync if j % 2 else nc.gpsimd).dma_start(
                    wq_j, t["wq"].ap()[j])
                wqs.append(wq_j)

            # PSUM discipline: tags "a"/"b" hold ONLY the long-lived oa/ob
            # accumulators; every transient (pkv/tp/pswk/psq/psw/rbp/psy)
            # lives in the short-lived "st" tag rotation, so interleaving
            # never aliases a live accumulator bank.
            # kv chain split in two emission halves (software pipeline):
            # the next l-block's dependency-free projection matmuls slot
            # between this block's matmuls and its DVE-dependent tail
            def kv_mm(l):
                xkv = xkvs[l]
                pkv = psst.tile([128, LQ], F32, tag="st", name="pkv")
                for f in range(FCH):
                    nc.tensor.matmul(pkv, wkv_sb[:, f, :], xkv[:, f, :],
                                     start=(f == 0), stop=(f == FCH - 1))
                kvl = kvp.tile([128, LQ], BF16, tag="kv", bufs=2)
                nc.scalar.activation(kvl, pkv,
                                     mybir.ActivationFunctionType.Identity,
                                     bias=bkv_sb[:, 0:1])
                lsl = ts(l, LQ)
                tmk = rtp.tile([D, LQ], BF16, tag="ksin")
                nc.vector.tensor_mul(tmk, kvl[0:64], sk[:, lsl])
                kc = rtp.tile([D, LQ], BF16, tag="kcos")
                nc.vector.tensor_mul(kc, kvl[0:64], ck[:, lsl])
                return l, kvl, tmk, kc

            def kv_fin(l, kvl, tmk, kc):
                # RoPE on k rows 0:64: krot = k*cos + Swap @ (k*sin_signed)
                lsl = ts(l, LQ)
                pswk = psst.tile([128, LQ], F32, tag="st", name="pswk")
                nc.tensor.matmul(pswk[0:64], swp[0:64, 0:64], tmk,
                                 start=True, stop=True)
                nc.vector.tensor_add(ktop[0:64, lsl], kc, pswk[0:64])
                nc.gpsimd.dma_start(kbot[64:128, lsl], ktop[0:64, lsl])

                # V transpose into vaug chunks (+ copy on idle ACT engine)
                for ci in range(4):
                    c = 4 * l + ci
                    tp = psst.tile([128, LQ], BF16, tag="st", name="tp")
                    nc.tensor.transpose(tp[:, 0:64], kvl[64:128, ts(ci, 128)],
                                        idtb[64:128, 64:128])
                    nc.scalar.copy(vaug[:, c, 0:D], tp[:, 0:64])

            # q_proj in two emission halves so the PE never sits behind a
            # DVE round-trip: the projection matmuls are dependency-free,
            # the swap matmul is emitted chunks later when tmq is ready
            def q_proj_mm(j):
                psq = psst.tile([128, LQ], F32, tag="st", name="psq")
                for f in range(FCH):
                    nc.tensor.matmul(psq, wqs[j][:, f, :], xq[:, f, :],
                                     start=(f == 0), stop=(f == FCH - 1))
                qraw = qrp.tile([128, LQ], BF16, tag="q")
                # bias on ACT (Identity+bias): frees psq's PSUM slot right
                # away instead of queueing behind DVE's mask backlog
                nc.scalar.activation(qraw, psq,
                                     mybir.ActivationFunctionType.Identity,
                                     bias=bq_sb[:, j:j + 1])
                tmq = rtp.tile([128, LQ], BF16, tag="qsin")
                nc.vector.tensor_mul(tmq, qraw, sq)
                qc = rtp.tile([128, LQ], BF16, tag="qcos")
                nc.vector.tensor_mul(qc, qraw, cq)
                return tmq, qc

            def q_proj_fin(j, tmq, qc):
                psw = psst.tile([128, LQ], F32, tag="st", name="psw")
                nc.tensor.matmul(psw, swp, tmq, start=True, stop=True)
                nc.vector.tensor_add(qrot[:, j, :], qc, psw)

            def q_proj(j):
                q_proj_fin(j, *q_proj_mm(j))

            prev_kv = None
            for l in range(NL):
                cur = kv_mm(l)
                if prev_kv is not None:
                    kv_fin(*prev_kv)
                prev_kv = cur
            kv_fin(*prev_kv)
            q_proj(0)

            # ================= phase C: attention =================
            onesh = small.tile([1, D], FP16, tag="onesh")
            nc.vector.memset(onesh, 1.0)

            def make_epilogue(j, oa, ob, last=False):
                """Normalize pair j's O accumulators. Returned as staged
                closures run inside pair j+1's chunk loop so nothing here
                sits on any engine's critical path. The reciprocal reads
                the PSUM denominator rows directly (f32), is bitcast to
                f32r for a K=1 ones-matmul broadcast down 64 partitions
                (rbp shares the st tag's PSUM buffers), then one DVE copy
                to SBUF feeds the two normalize muls."""
                den = recp.tile([1, 2, LQ], F32, tag="den")
                rcf = recp.tile([1, 2, LQ], F32, tag="rcf")
                rch = recp.tile([1, 2, LQ], FP16, tag="rch")
                rbs = recp.tile([D, 2, LQ], FP16, tag="rbs")
                osb = recp.tile([D, LQ], BF16, tag="osb")
                state = {}

                def s_den():
                    # custom-DVE ops can't address PSUM; stage via SBUF
                    nc.vector.tensor_copy(den[:, 0, :], oa[D:D + 1, :])
                    nc.vector.tensor_copy(den[:, 1, :], ob[D:D + 1, :])

                def s_recip():
                    nc.vector.reciprocal_approx_fast(rcf, den)

                def s_cast():
                    # fp16 so the broadcast matmul streams at 1 cyc/row
                    # (an fp32 matmul lowers to TWO half-rate PE passes);
                    # gpsimd keeps it off the busier DVE queue mid-stream,
                    # but the tail wants the low-latency DVE version
                    (nc.vector if last else nc.gpsimd).tensor_copy(rch, rcf)

                def s_bcast():
                    rbp = psst.tile([128, 2, LQ], F32, tag="st")
                    for tt in range(2):
                        nc.tensor.matmul(rbp[0:D, tt, :], onesh,
                                         rch[0:1, tt, :],
                                         start=True, stop=True)
                    state["rbp"] = rbp

                def s_copy():
                    nc.vector.tensor_copy(rbs, state["rbp"][0:D, :, :])

                def s_mul_a():
                    nc.vector.tensor_mul(obig[0:D, j, :], oa[0:D, :],
                                         rbs[:, 0, :])

                def s_mul_b():
                    nc.vector.tensor_mul(osb, ob[0:D, :], rbs[:, 1, :])
                    nc.gpsimd.dma_start(obig[64:128, j, :], osb)

                return {2: s_den, 3: s_recip, 4: s_cast, 8: s_bcast,
                        9: s_copy, 11: s_mul_a, 12: s_mul_b}

            # Flat (pair, chunk) stream with the S matmuls running one
            # chunk ahead of the O matmuls — continuous across pair
            # boundaries, so the in-order PE queue never drains behind
            # the exp->mask round-trip and the p-state stays ramped.
            def emit_s(j, c):
                st = psst.tile([128, 2, LQ], F32, tag="st")
                nc.tensor.matmul(st[:, 0, :], ktop[:, ts(c, 128)],
                                 qrot[:, j, :], start=True, stop=True)
                nc.tensor.matmul(st[:, 1, :], kbot[:, ts(c, 128)],
                                 qrot[:, j, :], start=True, stop=True)
                return st

            seq = [(j, c) for j in range(PAIRS) for c in range(KCH)]
            sts = {seq[0]: emit_s(*seq[0])}
            oab = {}
            pend = {}
            wos = []
            for i, (j, c) in enumerate(seq):
                if c == 0:
                    oab[j] = (psa.tile([128, LQ], F32, tag="a", name="oa"),
                              psb.tile([128, LQ], F32, tag="b", name="ob"))

                if i + 1 < len(seq):
                    sts[seq[i + 1]] = emit_s(*seq[i + 1])
                st = sts.pop((j, c))
                pt = ptp.tile([128, 2, LQ], FP16, tag="p")
                nc.scalar.activation(pt, st,
                                     mybir.ActivationFunctionType.Exp)
                # two chunks per pair go to gpsimd, timed to when DVE is
                # busy with the staged epilogue of the previous pair
                meng = nc.gpsimd if c in (3, 4) else nc.vector
                meng.tensor_mul(pt[:, :, :], pt[:, :, :],
                                mt2[:, c, :, :])
                oa, ob = oab[j]
                nc.tensor.matmul(oa[0:D + 1, :], vaug[:, c, :],
                                 pt[:, 0, :], start=(c == 0),
                                 stop=(c == KCH - 1))
                nc.tensor.matmul(ob[0:D + 1, :], vaug[:, c, :],
                                 pt[:, 1, :], start=(c == 0),
                                 stop=(c == KCH - 1))
                if c in pend:
                    pend.pop(c)()          # staged epilogue of pair j-1
                if j + 1 < PAIRS:
                    # next pair's Q projection, split across PE slack slots
                    if c == 6:
                        qnext = q_proj_mm(j + 1)
                    elif c == 10:
                        q_proj_fin(j + 1, *qnext)
                if j == 6 and c == 0:
                    for fb in range(FCH):
                        wo_fb = wst.tile([128, FCH, 128], BF16, tag="wo",
                                         bufs=FCH, name="wo_fb")
                        nc.gpsimd.dma_start(wo_fb, t["wo"].ap()[fb])
                        wos.append(wo_fb)
                if c == KCH - 1:
                    pend = make_epilogue(j, oa, ob, last=(j == PAIRS - 1))
                    oab.pop(j)
            for c in sorted(pend):
                pend[c]()                  # last pair's epilogue

            # ================= phase D: output projection =================
            for fb in range(FCH):
                psy = psa.tile([128, LQ], F32, tag="a")
                for j in range(FCH):
                    nc.tensor.matmul(psy, wos[fb][:, j, :], obig[:, j, :],
                                     start=(j == 0), stop=(j == FCH - 1))
                ysb = yout.tile([128, LQ], F32, tag="y")
                nc.vector.tensor_scalar_add(ysb, psy, bo_sb[:, fb:fb + 1])
                nc.sync.dma_start(yT.ap()[ts(fb, 128), :], ysb)

    nc.compile()
    return nc


def _tables():
    """RoPE tables in halves-permuted basis: rows i (even-half) hold +sin,
    rows 32+i (odd-half) hold -sin (for the tmp-then-swap formulation)."""
    inv_freq = 1.0 / (10000.0 ** (np.arange(0, D, 2, dtype=np.float64) / D))
    ang = np.outer(inv_freq, np.arange(L, dtype=np.float64))  # [32, L]
    cos = np.cos(ang).astype(np.float32)
    sin = np.sin(ang).astype(np.float32)
    cos64 = np.concatenate([cos, cos], axis=0)                # [64, L]
    sin_sgn = np.concatenate([sin, -sin], axis=0)             # [64, L]
    return cos64, sin_sgn


def _prep_weights(Wq, bq, Wk, bk, Wv, bv, Wo, bo):
    perm = np.concatenate([np.arange(0, D, 2), np.arange(1, D, 2)])
    WqP = np.asarray(Wq, dtype=np.float32)[:, :, perm].reshape(F, H * D)
    bqP = np.asarray(bq, dtype=np.float32)[:, perm].reshape(H * D)
    WkP = np.asarray(Wk, dtype=np.float32)[:, perm]
    bkP = np.asarray(bk, dtype=np.float32)[perm]
    Wkv = np.concatenate([WkP, np.asarray(Wv, dtype=np.float32)], axis=1)
    bkv = np.concatenate([bkP, np.asarray(bv, dtype=np.float32)])
    WoR = np.asarray(Wo, dtype=np.float32).reshape(H * D, F)
    bo_ = np.asarray(bo, dtype=np.float32)

    wq_pret = np.ascontiguousarray(
        WqP.reshape(FCH, 128, FCH, 128).transpose(2, 1, 0, 3)).astype(
            ml_dtypes.bfloat16)
    wkv_pret = np.ascontiguousarray(
        Wkv.reshape(FCH, 128, 128).transpose(1, 0, 2)).astype(
            ml_dtypes.bfloat16)
    wo_pret = np.ascontiguousarray(
        WoR.reshape(FCH, 128, FCH, 128).transpose(2, 1, 0, 3)).astype(
            ml_dtypes.bfloat16)
    bqbo = np.ascontiguousarray(np.concatenate(
        [bqP.reshape(FCH, 128).T, bo_.reshape(FCH, 128).T], axis=1))
    return wq_pret, wkv_pret, wo_pret, bqbo, bkv


def kernel(inputs_q, inputs_kv, mask, Wq, bq, Wk, bk, Wv, bv, Wo, bo):
    if "nc" not in _CACHED:
        _CACHED["nc"] = build_nc()
    nc = _CACHED["nc"]

    wq_pret, wkv_pret, wo_pret, bqbo, bkv = _prep_weights(
        Wq, bq, Wk, bk, Wv, bv, Wo, bo)

    cos64, sin_sgn = _tables()
    scale = 1.0 / np.sqrt(np.float32(D))
    cksk = np.ascontiguousarray(
        np.concatenate([cos64, sin_sgn], axis=1))      # [64, 2*L] (L=LK)
    cosq_full = np.tile(cos64 * scale, (2, 1))         # [128, L]
    sinq_full = np.tile(sin_sgn * scale, (2, 1))

    xq = np.asarray(inputs_q, dtype=np.float32)
    xkv = np.asarray(inputs_kv, dtype=np.float32)
    mk = np.asarray(mask)

    in_maps = []
    for core in range(NCORES):
        b = core // 4
        qs = (core % 4) * LQ
        xq_t = np.ascontiguousarray(
            xq[b, qs:qs + LQ, :].T.reshape(FCH, 128, LQ)
            .transpose(1, 0, 2)).astype(ml_dtypes.bfloat16)
        xkv_t = np.ascontiguousarray(
            xkv[b].T.reshape(FCH, 128, NL, LQ).transpose(2, 1, 0, 3)).astype(
                ml_dtypes.bfloat16)
        mask_1 = (mk[b, 0, qs:qs + LQ, :].T.reshape(KCH, 128, LQ)
                  .transpose(1, 0, 2).astype(np.float16))    # [p, c, lq]
        mask_t = np.ascontiguousarray(
            np.broadcast_to(mask_1[:, :, None, :], (128, KCH, 2, LQ)))
        in_maps.append({
            "xq_t": xq_t,
            "xkv_t": xkv_t,
            "mask_t": mask_t,
            "wq": wq_pret,
            "wkv": wkv_pret,
            "wo": wo_pret,
            "bqbo": bqbo,
            "bkv": bkv,
            "cosq": np.ascontiguousarray(
                cosq_full[:, qs:qs + LQ]).astype(ml_dtypes.bfloat16),
            "sinq": np.ascontiguousarray(
                sinq_full[:, qs:qs + LQ]).astype(ml_dtypes.bfloat16),
            "cksk": cksk.astype(ml_dtypes.bfloat16),
        })

    res = bass_utils.run_bass_kernel_spmd(nc, in_maps,
                                          core_ids=list(range(NCORES)))
    _CACHED["last_results"] = res
    _CACHED["last_maps"] = in_maps

    out = np.empty((B, L, F), dtype=np.float32)
    for core in range(NCORES):
        b = core // 4
        qs = (core % 4) * LQ
        out[b, qs:qs + LQ, :] = res.results[core]["yT"].T
    return out


# revision 73
# speedup vs baseline: 1.2452x; 1.2452x over previous
"""MQA attention (B=2, Lq=Lkv=2048, F=1024, H=16, D=64) on 8 TRN2 cores.

Sharding: core = (batch, query-block-of-512). Each core computes its full
output rows (all 16 heads + output projection) -> no collectives; host
concatenates per-core yT slabs.

v2 dataflow (bf16/fp16 matmul operands, f32 PSUM accumulation):
  KV chain first (per 512-row l-block): kvT = Wkv.T @ xkvT -> RoPE-k
  (halves-permuted basis, swap via small PE matmul) -> ktop/kbot bf16;
  V transposed into vaug fp16 (ones col 64 = softmax denominator row).
  Q proj per head-pair j (interleaved into the attention pair loop):
  qT = Wq_j.T @ xqT -> RoPE -> qrot bf16.
  Attention per (pair j, kv-chunk c): S^T x2 (ktop/kbot stationary) ->
  exp on ACT ([128,2,512] PSUM supertile -> fp16) -> mask mul x2 on DVE
  (fp16 2x mode) -> O accumulation x2 (vaug stationary).
  Pair epilogue off the PE critical path: denominator rows -> DVE
  reciprocal_approx_fast -> fp16 -> K=1 ones matmul broadcast -> DVE
  normalize muls -> obig bf16 (head 2j+1 half moved down via gpsimd DMA).
  yT = Wo-chunks.T @ obig (+bo), Wo fully prefetched during attention.
"""

import ml_dtypes
import numpy as np

import concourse.bass as bass
import concourse.tile as tile
from concourse import bacc, mybir
from concourse import bass_utils
from concourse.bass import ts
from concourse.masks import make_identity

F32 = mybir.dt.float32
BF16 = mybir.dt.bfloat16
FP16 = mybir.dt.float16

B, L, F, H, D = 2, 2048, 1024, 16, 64
LQ = 512            # query rows per core
LK = 2048           # kv rows (full)
NCORES = 8
PAIRS = H // 2      # head pairs (one qT partition block each)
FCH = F // 128      # f contraction chunks
KCH = LK // 128     # lk chunks
NL = LK // LQ       # kv l-blocks

_CACHED = {}


def build_nc():
    nc = bacc.Bacc("TRN2", target_bir_lowering=False, debug=False,
                   num_devices=NCORES)
    dt_in = [
        ("xq_t", [128, FCH, LQ], BF16),        # [p, f, lq] - 8KB DMA lines
        ("xkv_t", [NL, 128, FCH, LQ], BF16),   # [l, p, f, lq]
        ("mask_t", [128, KCH, 2, LQ], FP16),   # [p, c, tt, lq] pre-duplicated
        ("wq", [FCH, 128, FCH, 128], BF16),    # [j, p, f, m]
        ("wkv", [128, FCH, 128], BF16),        # [p, f, m]
        ("wo", [FCH, 128, FCH, 128], BF16),    # [fb, p, j, m]
        ("bqbo", [128, 2 * FCH], F32),         # cols 0:8 bq-blocks, 8:16 bo
        ("bkv", [2 * D], F32),
        ("cosq", [128, LQ], BF16),
        ("sinq", [128, LQ], BF16),
        ("cksk", [D, 2 * LK], BF16),           # [p, (cos|sin)*lk]
    ]
    t = {name: nc.dram_tensor(name, shape, dt, kind="ExternalInput")
         for name, shape, dt in dt_in}
    yT = nc.dram_tensor("yT", [F, LQ], F32, kind="ExternalOutput")

    with tile.TileContext(nc) as tc:
        with (
            tc.tile_pool(name="persist", bufs=1) as persist,
            tc.tile_pool(name="ptiles", bufs=3) as ptp,
            tc.tile_pool(name="small", bufs=4) as small,
            tc.tile_pool(name="xin", bufs=2) as xin,
            tc.tile_pool(name="wst", bufs=2) as wst,
            tc.tile_pool(name="qraw", bufs=2) as qrp,
            tc.tile_pool(name="kvraw", bufs=2) as kvp,
            tc.tile_pool(name="ropetmp", bufs=2) as rtp,
            tc.tile_pool(name="rec", bufs=2) as recp,
            tc.tile_pool(name="yout", bufs=2) as yout,
            tc.tile_pool(name="psa", bufs=2, space="PSUM") as psa,   # 2 banks
            tc.tile_pool(name="psb", bufs=2, space="PSUM") as psb,   # 2 banks
            tc.tile_pool(name="psst", bufs=2, space="PSUM") as psst,  # 4 banks
        ):
            # ---------------- small constants (gpsimd DMA queue) ---------
            cq = persist.tile([128, LQ], BF16)
            sq = persist.tile([128, LQ], BF16)
            cksk = persist.tile([D, 2, LK], BF16)
            # tables lead the scalar queue (ACT is idle in the lead-in;
            # each engine queue has its own DMA descriptor pipe, so
            # spreading input DMAs across queues parallelizes transfers)
            nc.scalar.dma_start(cq, t["cosq"].ap())
            nc.scalar.dma_start(sq, t["sinq"].ap())
            nc.scalar.dma_start(cksk,
                                t["cksk"].ap().rearrange("p (a l) -> p a l",
                                                         a=2))
            ck = cksk[:, 0, :]
            sk = cksk[:, 1, :]
            bqbo = small.tile([128, 2 * FCH], F32, tag="bias")
            nc.scalar.dma_start(bqbo, t["bqbo"].ap())
            bq_sb = bqbo[:, 0:FCH]
            bo_sb = bqbo[:, FCH:2 * FCH]
            bkv_sb = small.tile([128, 1], F32, tag="bias2")
            nc.scalar.dma_start(bkv_sb, t["bkv"].ap().unsqueeze(1))

            # mask chunks, host-duplicated per head-half: the pt multiply
            # is a single contiguous free-size-1024 DVE op AND the DMA
            # moves 8KB-contiguous per-partition lines.
            mt2 = persist.tile([128, KCH, 2, LQ], FP16)

            # ---------------- persistent compute tiles -------------------
            qrot = persist.tile([128, PAIRS, LQ], BF16)
            ktop = persist.tile([128, LK], BF16)          # k rows 0:64
            kbot = persist.tile([128, LK], BF16)          # k rows 64:128
            vaug = persist.tile([128, KCH, D + 1], FP16)  # V chunks + ones col
            obig = persist.tile([128, PAIRS, LQ], BF16)   # normalized O^T

            idt = small.tile([128, 128], F32, tag="ident")
            make_identity(nc, idt)
            idtb = small.tile([128, 128], BF16, tag="identb")
            nc.vector.tensor_copy(idtb, idt)
            # halves-swap permutation matrix: M[p, p-xor-32-within-head] = 1
            swpf = small.tile([128, 128], F32, tag="swpf")
            nc.gpsimd.memset(swpf, 0.0)
            for o1, o2 in ((0, 32), (32, 0), (64, 96), (96, 64)):
                nc.gpsimd.affine_select(
                    out=swpf[o1:o1 + 32, o2:o2 + 32],
                    in_=swpf[o1:o1 + 32, o2:o2 + 32],
                    compare_op=mybir.AluOpType.not_equal, fill=1.0,
                    base=0, pattern=[[-1, 32]], channel_multiplier=1)
            swp = small.tile([128, 128], BF16, tag="swp")
            nc.vector.tensor_copy(swp, swpf)
            nc.vector.memset(ktop[64:128], 0.0)
            nc.vector.memset(kbot[0:64], 0.0)
            nc.vector.memset(vaug[:, :, D:D + 1], 1.0)

            # ======== phase A/B: KV chain + Q projections, interleaved ====
            # Sync-queue DMA order IS the bandwidth priority order: wkv,
            # xkv_l0, xq, wq0, xkv_l1, wq1, ... One batched DMA per block
            # (DMA issue on the queue engine costs ~600ns per instruction).
            wkv_sb = wst.tile([128, FCH, 128], BF16, tag="wkv")
            nc.sync.dma_start(wkv_sb, t["wkv"].ap())
            xq = persist.tile([128, FCH, LQ], BF16)
            xkvs = []
            wqs = []
            # xkv in halves for finer matmul deps, round-robined over
            # queues so the per-queue descriptor pipes run in parallel
            kvq = [nc.sync, nc.gpsimd, nc.sync, nc.gpsimd,
                   nc.sync, nc.gpsimd, nc.sync, nc.gpsimd]
            for l in range(NL):
                xkv = xin.tile([128, FCH, LQ], BF16, tag="x", bufs=NL)
                for h in range(2):
                    kvq[2 * l + h].dma_start(
                        xkv[:, 4 * h:4 * h + 4, :],
                        t["xkv_t"].ap()[l][:, 4 * h:4 * h + 4, :])
                xkvs.append(xkv)
                if l == 0:
                    nc.scalar.dma_start(xq, t["xq_t"].ap())
                    wq_0 = wst.tile([128, FCH, 128], BF16, tag="wq", bufs=8)
                    nc.scalar.dma_start(wq_0, t["wq"].ap()[0])
                    wqs.append(wq_0)
            # mask stream in quarter granularity so pair-0 chunk c only
            # waits on its own quarter
            mq_eng = [nc.scalar, nc.gpsimd, nc.sync, nc.scalar]
            for mq in range(4):
                mq_eng[mq].dma_start(
                    mt2[:, 4 * mq:4 * mq + 4, :, :],
                    t["mask_t"].ap()[:, 4 * mq:4 * mq + 4, :, :])
            for j in range(1, PAIRS):
                wq_j = wst.tile([128, FCH, 128], BF16, tag="wq", bufs=8)
                (nc.sync if j % 2 else nc.gpsimd).dma_start(
                    wq_j, t["wq"].ap()[j])
                wqs.append(wq_j)

            # PSUM discipline: tags "a"/"b" hold ONLY the long-lived oa/ob
            # accumulators; every transient (pkv/tp/pswk/psq/psw/rbp/psy)
            # lives in the short-lived "st" tag rotation, so interleaving
            # never aliases a live accumulator bank.
            # kv chain split in two emission halves (software pipeline):
            # the next l-block's dependency-free projection matmuls slot
            # between this block's matmuls and its DVE-dependent tail
            def kv_mm(l):
                xkv = xkvs[l]
                pkv = psst.tile([128, LQ], F32, tag="st", name="pkv")
                for f in range(FCH):
                    nc.tensor.matmul(pkv, wkv_sb[:, f, :], xkv[:, f, :],
                                     start=(f == 0), stop=(f == FCH - 1))
                kvl = kvp.tile([128, LQ], BF16, tag="kv", bufs=2)
                nc.scalar.activation(kvl, pkv,
                                     mybir.ActivationFunctionType.Identity,
                                     bias=bkv_sb[:, 0:1])
                lsl = ts(l, LQ)
                tmk = rtp.tile([D, LQ], BF16, tag="ksin")
                nc.vector.tensor_mul(tmk, kvl[0:64], sk[:, lsl])
                kc = rtp.tile([D, LQ], BF16, tag="kcos")
                nc.vector.tensor_mul(kc, kvl[0:64], ck[:, lsl])
                return l, kvl, tmk, kc

            def kv_fin(l, kvl, tmk, kc):
                # RoPE on k rows 0:64: krot = k*cos + Swap @ (k*sin_signed)
                lsl = ts(l, LQ)
                pswk = psst.tile([128, LQ], F32, tag="st", name="pswk")
                nc.tensor.matmul(pswk[0:64], swp[0:64, 0:64], tmk,
                                 start=True, stop=True)
                nc.vector.tensor_add(ktop[0:64, lsl], kc, pswk[0:64])
                nc.gpsimd.dma_start(kbot[64:128, lsl], ktop[0:64, lsl])

                # V transpose into vaug chunks (+ copy on idle ACT engine)
                for ci in range(4):
                    c = 4 * l + ci
                    tp = psst.tile([128, LQ], BF16, tag="st", name="tp")
                    nc.tensor.transpose(tp[:, 0:64], kvl[64:128, ts(ci, 128)],
                                        idtb[64:128, 64:128])
                    nc.scalar.copy(vaug[:, c, 0:D], tp[:, 0:64])

            # q_proj in two emission halves so the PE never sits behind a
            # DVE round-trip: the projection matmuls are dependency-free,
            # the swap matmul is emitted chunks later when tmq is ready
            def q_proj_mm(j):
                psq = psst.tile([128, LQ], F32, tag="st", name="psq")
                for f in range(FCH):
                    nc.tensor.matmul(psq, wqs[j][:, f, :], xq[:, f, :],
                                     start=(f == 0), stop=(f == FCH - 1))
                qraw = qrp.tile([128, LQ], BF16, tag="q")
                # bias on ACT (Identity+bias): frees psq's PSUM slot right
                # away instead of queueing behind DVE's mask backlog
                nc.scalar.activation(qraw, psq,
                                     mybir.ActivationFunctionType.Identity,
                                     bias=bq_sb[:, j:j + 1])
                tmq = rtp.tile([128, LQ], BF16, tag="qsin")
                nc.vector.tensor_mul(tmq, qraw, sq)
                qc = rtp.tile([128, LQ], BF16, tag="qcos")
                nc.vector.tensor_mul(qc, qraw, cq)
                return tmq, qc

            def q_proj_fin(j, tmq, qc):
                psw = psst.tile([128, LQ], F32, tag="st", name="psw")
                nc.tensor.matmul(psw, swp, tmq, start=True, stop=True)
                nc.vector.tensor_add(qrot[:, j, :], qc, psw)

            def q_proj(j):
                q_proj_fin(j, *q_proj_mm(j))

            prev_kv = None
            for l in range(NL):
                cur = kv_mm(l)
                if prev_kv is not None:
                    kv_fin(*prev_kv)
                prev_kv = cur
            kv_fin(*prev_kv)
            q_proj(0)

            # ================= phase C: attention =================
            onesh = small.tile([1, D], FP16, tag="onesh")
            nc.vector.memset(onesh, 1.0)

            def make_epilogue(j, oa, ob, last=False):
                """Normalize pair j's O accumulators. Returned as staged
                closures run inside pair j+1's chunk loop so nothing here
                sits on any engine's critical path. The reciprocal reads
                the PSUM denominator rows directly (f32), is bitcast to
                f32r for a K=1 ones-matmul broadcast down 64 partitions
                (rbp shares the st tag's PSUM buffers), then one DVE copy
                to SBUF feeds the two normalize muls."""
                den = recp.tile([1, 2, LQ], F32, tag="den")
                rcf = recp.tile([1, 2, LQ], F32, tag="rcf")
                rch = recp.tile([1, 2, LQ], FP16, tag="rch")
                rbs = recp.tile([D, 2, LQ], FP16, tag="rbs")
                osb = recp.tile([D, LQ], BF16, tag="osb")
                state = {}

                def s_den():
                    # custom-DVE ops can't address PSUM; stage via SBUF
                    nc.vector.tensor_copy(den[:, 0, :], oa[D:D + 1, :])
                    nc.vector.tensor_copy(den[:, 1, :], ob[D:D + 1, :])

                def s_recip():
                    nc.vector.reciprocal_approx_fast(rcf, den)

                def s_cast():
                    # fp16 so the broadcast matmul streams at 1 cyc/row
                    # (an fp32 matmul lowers to TWO half-rate PE passes);
                    # gpsimd keeps it off the busier DVE queue mid-stream,
                    # but the tail wants the low-latency DVE version
                    (nc.vector if last else nc.gpsimd).tensor_copy(rch, rcf)

                def s_bcast():
                    rbp = psst.tile([128, 2, LQ], F32, tag="st")
                    for tt in range(2):
                        nc.tensor.matmul(rbp[0:D, tt, :], onesh,
                                         rch[0:1, tt, :],
                                         start=True, stop=True)
                    state["rbp"] = rbp

                def s_copy():
                    nc.vector.tensor_copy(rbs, state["rbp"][0:D, :, :])

                def s_mul_a():
                    nc.vector.tensor_mul(obig[0:D, j, :], oa[0:D, :],
                                         rbs[:, 0, :])

                def s_mul_b():
                    nc.vector.tensor_mul(osb, ob[0:D, :], rbs[:, 1, :])
                    nc.gpsimd.dma_start(obig[64:128, j, :], osb)

                return {2: s_den, 3: s_recip, 4: s_cast, 8: s_bcast,
                        9: s_copy, 11: s_mul_a, 12: s_mul_b}

            # Flat (pair, chunk) stream with the S matmuls running one
            # chunk ahead of the O matmuls — continuous across pair
            # boundaries, so the in-order PE queue never drains behind
            # the exp->mask round-trip and the p-state stays ramped.
            def emit_s(j, c):
                st = psst.tile([128, 2, LQ], F32, tag="st")
                nc.tensor.matmul(st[:, 0, :], ktop[:, ts(c, 128)],
                                 qrot[:, j, :], start=True, stop=True)
                nc.tensor.matmul(st[:, 1, :], kbot[:, ts(c, 128)],
                                 qrot[:, j, :], start=True, stop=True)
                return st

            seq = [(j, c) for j in range(PAIRS) for c in range(KCH)]
            sts = {seq[0]: emit_s(*seq[0])}
            oab = {}
            pend = {}
            wos = []
            for i, (j, c) in enumerate(seq):
                if c == 0:
                    oab[j] = (psa.tile([128, LQ], F32, tag="a", name="oa"),
                              psb.tile([128, LQ], F32, tag="b", name="ob"))

                if i + 1 < len(seq):
                    sts[seq[i + 1]] = emit_s(*seq[i + 1])
                st = sts.pop((j, c))
                pt = ptp.tile([128, 2, LQ], FP16, tag="p")
                nc.scalar.activation(pt, st,
                                     mybir.ActivationFunctionType.Exp)
                nc.vector.tensor_mul(pt[:, :, :], pt[:, :, :],
                                     mt2[:, c, :, :])
                oa, ob = oab[j]
                nc.tensor.matmul(oa[0:D + 1, :], vaug[:, c, :],
                                 pt[:, 0, :], start=(c == 0),
                                 stop=(c == KCH - 1))
                nc.tensor.matmul(ob[0:D + 1, :], vaug[:, c, :],
                                 pt[:, 1, :], start=(c == 0),
                                 stop=(c == KCH - 1))
                if c in pend:
                    pend.pop(c)()          # staged epilogue of pair j-1
                if j + 1 < PAIRS:
                    # next pair's Q projection, split across PE slack slots
                    if c == 6:
                        qnext = q_proj_mm(j + 1)
                    elif c == 10:
                        q_proj_fin(j + 1, *qnext)
                if j == 6 and c == 0:
                    for fb in range(FCH):
                        wo_fb = wst.tile([128, FCH, 128], BF16, tag="wo",
                                         bufs=FCH, name="wo_fb")
                        nc.gpsimd.dma_start(wo_fb, t["wo"].ap()[fb])
                        wos.append(wo_fb)
                if c == KCH - 1:
                    pend = make_epilogue(j, oa, ob, last=(j == PAIRS - 1))
                    oab.pop(j)
            for c in sorted(pend):
                pend[c]()                  # last pair's epilogue

            # ================= phase D: output projection =================
            for fb in range(FCH):
                psy = psa.tile([128, LQ], F32, tag="a")
                for j in range(FCH):
                    nc.tensor.matmul(psy, wos[fb][:, j, :], obig[:, j, :],
                                     start=(j == 0), stop=(j == FCH - 1))
                ysb = yout.tile([128, LQ], F32, tag="y")
                nc.vector.tensor_scalar_add(ysb, psy, bo_sb[:, fb:fb + 1])
                nc.sync.dma_start(yT.ap()[ts(fb, 128), :], ysb)

    nc.compile()
    return nc


def _tables():
    """RoPE tables in halves-permuted basis: rows i (even-half) hold +sin,
    rows 32+i (odd-half) hold -sin (for the tmp-then-swap formulation)."""
    inv_freq = 1.0 / (10000.0 ** (np.arange(0, D, 2, dtype=np.float64) / D))
    ang = np.outer(inv_freq, np.arange(L, dtype=np.float64))  # [32, L]
    cos = np.cos(ang).astype(np.float32)
    sin = np.sin(ang).astype(np.float32)
    cos64 = np.concatenate([cos, cos], axis=0)                # [64, L]
    sin_sgn = np.concatenate([sin, -sin], axis=0)             # [64, L]
    return cos64, sin_sgn


def _prep_weights(Wq, bq, Wk, bk, Wv, bv, Wo, bo):
    perm = np.concatenate([np.arange(0, D, 2), np.arange(1, D, 2)])
    WqP = np.asarray(Wq, dtype=np.float32)[:, :, perm].reshape(F, H * D)
    bqP = np.asarray(bq, dtype=np.float32)[:, perm].reshape(H * D)
    WkP = np.asarray(Wk, dtype=np.float32)[:, perm]
    bkP = np.asarray(bk, dtype=np.float32)[perm]
    Wkv = np.concatenate([WkP, np.asarray(Wv, dtype=np.float32)], axis=1)
    bkv = np.concatenate([bkP, np.asarray(bv, dtype=np.float32)])
    WoR = np.asarray(Wo, dtype=np.float32).reshape(H * D, F)
    bo_ = np.asarray(bo, dtype=np.float32)

    wq_pret = np.ascontiguousarray(
        WqP.reshape(FCH, 128, FCH, 128).transpose(2, 1, 0, 3)).astype(
            ml_dtypes.bfloat16)
    wkv_pret = np.ascontiguousarray(
        Wkv.reshape(FCH, 128, 128).transpose(1, 0, 2)).astype(
            ml_dtypes.bfloat16)
    wo_pret = np.ascontiguousarray(
        WoR.reshape(FCH, 128, FCH, 128).transpose(2, 1, 0, 3)).astype(
            ml_dtypes.bfloat16)
    bqbo = np.ascontiguousarray(np.concatenate(
        [bqP.reshape(FCH, 128).T, bo_.reshape(FCH, 128).T], axis=1))
    return wq_pret, wkv_pret, wo_pret, bqbo, bkv


def kernel(inputs_q, inputs_kv, mask, Wq, bq, Wk, bk, Wv, bv, Wo, bo):
    if "nc" not in _CACHED:
        _CACHED["nc"] = build_nc()
    nc = _CACHED["nc"]

    wq_pret, wkv_pret, wo_pret, bqbo, bkv = _prep_weights(
        Wq, bq, Wk, bk, Wv, bv, Wo, bo)

    cos64, sin_sgn = _tables()
    scale = 1.0 / np.sqrt(np.float32(D))
    cksk = np.ascontiguousarray(
        np.concatenate([cos64, sin_sgn], axis=1))      # [64, 2*L] (L=LK)
    cosq_full = np.tile(cos64 * scale, (2, 1))         # [128, L]
    sinq_full = np.tile(sin_sgn * scale, (2, 1))

    xq = np.asarray(inputs_q, dtype=np.float32)
    xkv = np.asarray(inputs_kv, dtype=np.float32)
    mk = np.asarray(mask)

    in_maps = []
    for core in range(NCORES):
        b = core // 4
        qs = (core % 4) * LQ
        xq_t = np.ascontiguousarray(
            xq[b, qs:qs + LQ, :].T.reshape(FCH, 128, LQ)
            .transpose(1, 0, 2)).astype(ml_dtypes.bfloat16)
        xkv_t = np.ascontiguousarray(
            xkv[b].T.reshape(FCH, 128, NL, LQ).transpose(2, 1, 0, 3)).astype(
                ml_dtypes.bfloat16)
        mask_1 = (mk[b, 0, qs:qs + LQ, :].T.reshape(KCH, 128, LQ)
                  .transpose(1, 0, 2).astype(np.float16))    # [p, c, lq]
        mask_t = np.ascontiguousarray(
            np.broadcast_to(mask_1[:, :, None, :], (128, KCH, 2, LQ)))
        in_maps.append({
            "xq_t": xq_t,
            "xkv_t": xkv_t,
            "mask_t": mask_t,
            "wq": wq_pret,
            "wkv": wkv_pret,
            "wo": wo_pret,
            "bqbo": bqbo,
            "bkv": bkv,
            "cosq": np.ascontiguousarray(
                cosq_full[:, qs:qs + LQ]).astype(ml_dtypes.bfloat16),
            "sinq": np.ascontiguousarray(
                sinq_full[:, qs:qs + LQ]).astype(ml_dtypes.bfloat16),
            "cksk": cksk.astype(ml_dtypes.bfloat16),
        })

    res = bass_utils.run_bass_kernel_spmd(nc, in_maps,
                                          core_ids=list(range(NCORES)))
    _CACHED["last_results"] = res
    _CACHED["last_maps"] = in_maps

    out = np.empty((B, L, F), dtype=np.float32)
    for core in range(NCORES):
        b = core // 4
        qs = (core % 4) * LQ
        out[b, qs:qs + LQ, :] = res.results[core]["yT"].T
    return out


# revision 75
# speedup vs baseline: 1.2948x; 1.0399x over previous
"""MQA attention (B=2, Lq=Lkv=2048, F=1024, H=16, D=64) on 8 TRN2 cores.

Sharding: core = (batch, query-block-of-512). Each core computes its full
output rows (all 16 heads + output projection) -> no collectives; host
concatenates per-core yT slabs.

v2 dataflow (bf16/fp16 matmul operands, f32 PSUM accumulation):
  KV chain first (per 512-row l-block): kvT = Wkv.T @ xkvT -> RoPE-k
  (halves-permuted basis, swap via small PE matmul) -> ktop/kbot bf16;
  V transposed into vaug fp16 (ones col 64 = softmax denominator row).
  Q proj per head-pair j (interleaved into the attention pair loop):
  qT = Wq_j.T @ xqT -> RoPE -> qrot bf16.
  Attention per (pair j, kv-chunk c): S^T x2 (ktop/kbot stationary) ->
  exp on ACT ([128,2,512] PSUM supertile -> fp16) -> mask mul x2 on DVE
  (fp16 2x mode) -> O accumulation x2 (vaug stationary).
  Pair epilogue off the PE critical path: denominator rows -> DVE
  reciprocal_approx_fast -> fp16 -> K=1 ones matmul broadcast -> DVE
  normalize muls -> obig bf16 (head 2j+1 half moved down via gpsimd DMA).
  yT = Wo-chunks.T @ obig (+bo), Wo fully prefetched during attention.
"""

import ml_dtypes
import numpy as np

import concourse.bass as bass
import concourse.tile as tile
from concourse import bacc, mybir
from concourse import bass_utils
from concourse.bass import ts
from concourse.masks import make_identity

F32 = mybir.dt.float32
BF16 = mybir.dt.bfloat16
FP16 = mybir.dt.float16

B, L, F, H, D = 2, 2048, 1024, 16, 64
LQ = 512            # query rows per core
LK = 2048           # kv rows (full)
NCORES = 8
PAIRS = H // 2      # head pairs (one qT partition block each)
FCH = F // 128      # f contraction chunks
KCH = LK // 128     # lk chunks
NL = LK // LQ       # kv l-blocks

_CACHED = {}


def build_nc():
    nc = bacc.Bacc("TRN2", target_bir_lowering=False, debug=False,
                   num_devices=NCORES)
    dt_in = [
        ("xq_t", [128, FCH, LQ], BF16),        # [p, f, lq] - 8KB DMA lines
        ("xkv_t", [NL, 128, FCH, LQ], BF16),   # [l, p, f, lq]
        ("mask_t", [128, KCH, 2, LQ], FP16),   # [p, c, tt, lq] pre-duplicated
        ("wq", [FCH, 128, FCH, 128], BF16),    # [j, p, f, m]
        ("wkv", [128, FCH, 128], BF16),        # [p, f, m]
        ("wo", [FCH, 128, FCH, 128], BF16),    # [fb, p, j, m]
        ("bqbo", [128, 2 * FCH], F32),         # cols 0:8 bq-blocks, 8:16 bo
        ("bkv", [2 * D], F32),
        ("cosq", [128, LQ], BF16),
        ("sinq", [128, LQ], BF16),
        ("cksk", [D, 2 * LK], BF16),           # [p, (cos|sin)*lk]
    ]
    t = {name: nc.dram_tensor(name, shape, dt, kind="ExternalInput")
         for name, shape, dt in dt_in}
    yT = nc.dram_tensor("yT", [F, LQ], F32, kind="ExternalOutput")

    with tile.TileContext(nc) as tc:
        with (
            tc.tile_pool(name="persist", bufs=1) as persist,
            tc.tile_pool(name="ptiles", bufs=3) as ptp,
            tc.tile_pool(name="small", bufs=4) as small,
            tc.tile_pool(name="xin", bufs=2) as xin,
            tc.tile_pool(name="wst", bufs=2) as wst,
            tc.tile_pool(name="qraw", bufs=2) as qrp,
            tc.tile_pool(name="kvraw", bufs=2) as kvp,
            tc.tile_pool(name="ropetmp", bufs=2) as rtp,
            tc.tile_pool(name="rec", bufs=2) as recp,
            tc.tile_pool(name="yout", bufs=2) as yout,
            tc.tile_pool(name="psa", bufs=2, space="PSUM") as psa,   # 2 banks
            tc.tile_pool(name="psb", bufs=2, space="PSUM") as psb,   # 2 banks
            tc.tile_pool(name="psst", bufs=2, space="PSUM") as psst,  # 4 banks
        ):
            # ---------------- small constants (gpsimd DMA queue) ---------
            cq = persist.tile([128, LQ], BF16)
            sq = persist.tile([128, LQ], BF16)
            cksk = persist.tile([D, 2, LK], BF16)
            # tables lead the scalar queue (ACT is idle in the lead-in;
            # each engine queue has its own DMA descriptor pipe, so
            # spreading input DMAs across queues parallelizes transfers)
            nc.scalar.dma_start(cq, t["cosq"].ap())
            nc.scalar.dma_start(sq, t["sinq"].ap())
            nc.scalar.dma_start(cksk,
                                t["cksk"].ap().rearrange("p (a l) -> p a l",
                                                         a=2))
            ck = cksk[:, 0, :]
            sk = cksk[:, 1, :]
            bqbo = small.tile([128, 2 * FCH], F32, tag="bias")
            nc.scalar.dma_start(bqbo, t["bqbo"].ap())
            bq_sb = bqbo[:, 0:FCH]
            bo_sb = bqbo[:, FCH:2 * FCH]
            bkv_sb = small.tile([128, 1], F32, tag="bias2")
            nc.scalar.dma_start(bkv_sb, t["bkv"].ap().unsqueeze(1))

            # mask chunks, host-duplicated per head-half: the pt multiply
            # is a single contiguous free-size-1024 DVE op AND the DMA
            # moves 8KB-contiguous per-partition lines.
            mt2 = persist.tile([128, KCH, 2, LQ], FP16)

            # ---------------- persistent compute tiles -------------------
            qrot = persist.tile([128, PAIRS, LQ], BF16)
            ktop = persist.tile([128, LK], BF16)          # k rows 0:64
            kbot = persist.tile([128, LK], BF16)          # k rows 64:128
            vaug = persist.tile([128, KCH, D + 1], FP16)  # V chunks + ones col
            obig = persist.tile([128, PAIRS, LQ], BF16)   # normalized O^T

            idt = small.tile([128, 128], F32, tag="ident")
            make_identity(nc, idt)
            idtb = small.tile([128, 128], BF16, tag="identb")
            nc.vector.tensor_copy(idtb, idt)
            # halves-swap permutation matrix: M[p, p-xor-32-within-head] = 1
            swpf = small.tile([128, 128], F32, tag="swpf")
            nc.gpsimd.memset(swpf, 0.0)
            for o1, o2 in ((0, 32), (32, 0), (64, 96), (96, 64)):
                nc.gpsimd.affine_select(
                    out=swpf[o1:o1 + 32, o2:o2 + 32],
                    in_=swpf[o1:o1 + 32, o2:o2 + 32],
                    compare_op=mybir.AluOpType.not_equal, fill=1.0,
                    base=0, pattern=[[-1, 32]], channel_multiplier=1)
            swp = small.tile([128, 128], BF16, tag="swp")
            nc.vector.tensor_copy(swp, swpf)
            nc.vector.memset(ktop[64:128], 0.0)
            nc.vector.memset(kbot[0:64], 0.0)
            nc.vector.memset(vaug[:, :, D:D + 1], 1.0)

            # ======== phase A/B: KV chain + Q projections, interleaved ====
            # Sync-queue DMA order IS the bandwidth priority order: wkv,
            # xkv_l0, xq, wq0, xkv_l1, wq1, ... One batched DMA per block
            # (DMA issue on the queue engine costs ~600ns per instruction).
            wkv_sb = wst.tile([128, FCH, 128], BF16, tag="wkv")
            nc.sync.dma_start(wkv_sb, t["wkv"].ap())
            xq = persist.tile([128, FCH, LQ], BF16)
            xkvs = []
            wqs = []
            # xkv in halves for finer matmul deps, round-robined over
            # queues so the per-queue descriptor pipes run in parallel
            kvq = [nc.sync, nc.gpsimd, nc.sync, nc.gpsimd,
                   nc.sync, nc.gpsimd, nc.sync, nc.gpsimd]
            for l in range(NL):
                xkv = xin.tile([128, FCH, LQ], BF16, tag="x", bufs=NL)
                for h in range(2):
                    kvq[2 * l + h].dma_start(
                        xkv[:, 4 * h:4 * h + 4, :],
                        t["xkv_t"].ap()[l][:, 4 * h:4 * h + 4, :])
                xkvs.append(xkv)
                if l == 0:
                    nc.scalar.dma_start(xq, t["xq_t"].ap())
                    wq_0 = wst.tile([128, FCH, 128], BF16, tag="wq", bufs=8)
                    nc.scalar.dma_start(wq_0, t["wq"].ap()[0])
                    wqs.append(wq_0)
            # mask stream in quarter granularity so pair-0 chunk c only
            # waits on its own quarter
            mq_eng = [nc.scalar, nc.gpsimd, nc.sync, nc.scalar]
            for mq in range(4):
                mq_eng[mq].dma_start(
                    mt2[:, 4 * mq:4 * mq + 4, :, :],
                    t["mask_t"].ap()[:, 4 * mq:4 * mq + 4, :, :])
            for j in range(1, PAIRS):
                wq_j = wst.tile([128, FCH, 128], BF16, tag="wq", bufs=8)
                (nc.sync if j % 2 else nc.gpsimd).dma_start(
                    wq_j, t["wq"].ap()[j])
                wqs.append(wq_j)

            # PSUM discipline: tags "a"/"b" hold ONLY the long-lived oa/ob
            # accumulators; every transient (pkv/tp/pswk/psq/psw/rbp/psy)
            # lives in the short-lived "st" tag rotation, so interleaving
            # never aliases a live accumulator bank.
            # kv chain split in two emission halves (software pipeline):
            # the next l-block's dependency-free projection matmuls slot
            # between this block's matmuls and its DVE-dependent tail
            def kv_mm(l):
                xkv = xkvs[l]
                pkv = psst.tile([128, LQ], F32, tag="st", name="pkv")
                for f in range(FCH):
                    nc.tensor.matmul(pkv, wkv_sb[:, f, :], xkv[:, f, :],
                                     start=(f == 0), stop=(f == FCH - 1))
                kvl = kvp.tile([128, LQ], BF16, tag="kv", bufs=2)
                nc.scalar.activation(kvl, pkv,
                                     mybir.ActivationFunctionType.Identity,
                                     bias=bkv_sb[:, 0:1])
                lsl = ts(l, LQ)
                tmk = rtp.tile([D, LQ], BF16, tag="ksin")
                nc.vector.tensor_mul(tmk, kvl[0:64], sk[:, lsl])
                kc = rtp.tile([D, LQ], BF16, tag="kcos")
                nc.vector.tensor_mul(kc, kvl[0:64], ck[:, lsl])
                return l, kvl, tmk, kc

            def kv_fin(l, kvl, tmk, kc):
                # RoPE on k rows 0:64: krot = k*cos + Swap @ (k*sin_signed)
                lsl = ts(l, LQ)
                pswk = psst.tile([128, LQ], F32, tag="st", name="pswk")
                nc.tensor.matmul(pswk[0:64], swp[0:64, 0:64], tmk,
                                 start=True, stop=True)
                nc.vector.tensor_add(ktop[0:64, lsl], kc, pswk[0:64])
                nc.gpsimd.dma_start(kbot[64:128, lsl], ktop[0:64, lsl])

                # V transpose into vaug chunks (+ copy on idle ACT engine)
                for ci in range(4):
                    c = 4 * l + ci
                    tp = psst.tile([128, LQ], BF16, tag="st", name="tp")
                    nc.tensor.transpose(tp[:, 0:64], kvl[64:128, ts(ci, 128)],
                                        idtb[64:128, 64:128])
                    nc.scalar.copy(vaug[:, c, 0:D], tp[:, 0:64])

            # q_proj in two emission halves so the PE never sits behind a
            # DVE round-trip: the projection matmuls are dependency-free,
            # the swap matmul is emitted chunks later when tmq is ready
            def q_proj_mm(j):
                psq = psst.tile([128, LQ], F32, tag="st", name="psq")
                for f in range(FCH):
                    nc.tensor.matmul(psq, wqs[j][:, f, :], xq[:, f, :],
                                     start=(f == 0), stop=(f == FCH - 1))
                qraw = qrp.tile([128, LQ], BF16, tag="q")
                # bias on ACT (Identity+bias): frees psq's PSUM slot right
                # away instead of queueing behind DVE's mask backlog
                nc.scalar.activation(qraw, psq,
                                     mybir.ActivationFunctionType.Identity,
                                     bias=bq_sb[:, j:j + 1])
                tmq = rtp.tile([128, LQ], BF16, tag="qsin")
                nc.vector.tensor_mul(tmq, qraw, sq)
                qc = rtp.tile([128, LQ], BF16, tag="qcos")
                nc.vector.tensor_mul(qc, qraw, cq)
                return tmq, qc

            def q_proj_fin(j, tmq, qc):
                psw = psst.tile([128, LQ], F32, tag="st", name="psw")
                nc.tensor.matmul(psw, swp, tmq, start=True, stop=True)
                nc.vector.tensor_add(qrot[:, j, :], qc, psw)

            def q_proj(j):
                q_proj_fin(j, *q_proj_mm(j))

            prev_kv = None
            for l in range(NL):
                cur = kv_mm(l)
                if prev_kv is not None:
                    kv_fin(*prev_kv)
                prev_kv = cur
            kv_fin(*prev_kv)
            q_proj(0)

            # ================= phase C: attention =================
            onesf = small.tile([1, D], F32, tag="onesf")
            nc.vector.memset(onesf, 1.0)

            def make_epilogue(j, oa, ob, last=False):
                """Normalize pair j's O accumulators. Returned as staged
                closures run inside pair j+1's chunk loop so nothing here
                sits on any engine's critical path. The reciprocal reads
                the PSUM denominator rows directly (f32), is bitcast to
                f32r for a K=1 ones-matmul broadcast down 64 partitions
                (rbp shares the st tag's PSUM buffers), then one DVE copy
                to SBUF feeds the two normalize muls."""
                den = recp.tile([1, 2, LQ], F32, tag="den")
                rcf = recp.tile([1, 2, LQ], F32, tag="rcf")
                rbs = recp.tile([D, 2, LQ], FP16, tag="rbs")
                osb = recp.tile([D, LQ], BF16, tag="osb")
                state = {}

                def s_den():
                    # custom-DVE ops can't address PSUM; stage via SBUF
                    nc.vector.tensor_copy(den[:, 0, :], oa[D:D + 1, :])
                    nc.vector.tensor_copy(den[:, 1, :], ob[D:D + 1, :])

                def s_recip():
                    nc.vector.reciprocal_approx_fast(rcf, den)

                def s_bcast():
                    # fp32 matmul costs ~2 PE passes but fits the PE's
                    # per-pair slack and keeps DVE/gpsimd out of the chain
                    rbp = psst.tile([128, 2, LQ], F32, tag="st")
                    for tt in range(2):
                        nc.tensor.matmul(rbp[0:D, tt, :], onesf,
                                         rcf[0:1, tt, :],
                                         start=True, stop=True)
                    state["rbp"] = rbp

                def s_copy():
                    nc.vector.tensor_copy(rbs, state["rbp"][0:D, :, :])

                def s_mul_a():
                    nc.vector.tensor_mul(obig[0:D, j, :], oa[0:D, :],
                                         rbs[:, 0, :])

                def s_mul_b():
                    nc.vector.tensor_mul(osb, ob[0:D, :], rbs[:, 1, :])
                    nc.gpsimd.dma_start(obig[64:128, j, :], osb)

                return {2: s_den, 3: s_recip, 6: s_bcast,
                        7: s_copy, 9: s_mul_a, 10: s_mul_b}

            # Flat (pair, chunk) stream with the S matmuls running one
            # chunk ahead of the O matmuls — continuous across pair
            # boundaries, so the in-order PE queue never drains behind
            # the exp->mask round-trip and the p-state stays ramped.
            def emit_s(j, c):
                st = psst.tile([128, 2, LQ], F32, tag="st")
                nc.tensor.matmul(st[:, 0, :], ktop[:, ts(c, 128)],
                                 qrot[:, j, :], start=True, stop=True)
                nc.tensor.matmul(st[:, 1, :], kbot[:, ts(c, 128)],
                                 qrot[:, j, :], start=True, stop=True)
                return st

            seq = [(j, c) for j in range(PAIRS) for c in range(KCH)]
            sts = {seq[0]: emit_s(*seq[0])}
            oab = {}
            pend = {}
            wos = []
            for i, (j, c) in enumerate(seq):
                if c == 0:
                    oab[j] = (psa.tile([128, LQ], F32, tag="a", name="oa"),
                              psb.tile([128, LQ], F32, tag="b", name="ob"))

                if i + 1 < len(seq):
                    sts[seq[i + 1]] = emit_s(*seq[i + 1])
                st = sts.pop((j, c))
                pt = ptp.tile([128, 2, LQ], FP16, tag="p")
                nc.scalar.activation(pt, st,
                                     mybir.ActivationFunctionType.Exp)
                nc.vector.tensor_mul(pt[:, :, :], pt[:, :, :],
                                     mt2[:, c, :, :])
                oa, ob = oab[j]
                nc.tensor.matmul(oa[0:D + 1, :], vaug[:, c, :],
                                 pt[:, 0, :], start=(c == 0),
                                 stop=(c == KCH - 1))
                nc.tensor.matmul(ob[0:D + 1, :], vaug[:, c, :],
                                 pt[:, 1, :], start=(c == 0),
                                 stop=(c == KCH - 1))
                if c in pend:
                    pend.pop(c)()          # staged epilogue of pair j-1
                if j + 1 < PAIRS:
                    # next pair's Q projection, split across PE slack slots
                    if c == 6:
                        qnext = q_proj_mm(j + 1)
                    elif c == 10:
                        q_proj_fin(j + 1, *qnext)
                if j == 6 and c == 0:
                    for fb in range(FCH):
                        wo_fb = wst.tile([128, FCH, 128], BF16, tag="wo",
                                         bufs=FCH, name="wo_fb")
                        nc.gpsimd.dma_start(wo_fb, t["wo"].ap()[fb])
                        wos.append(wo_fb)
                if c == KCH - 1:
                    pend = make_epilogue(j, oa, ob, last=(j == PAIRS - 1))
                    oab.pop(j)
            for c in sorted(pend):
                pend[c]()                  # last pair's epilogue

            # ================= phase D: output projection =================
            for fb in range(FCH):
                psy = psa.tile([128, LQ], F32, tag="a")
                for j in range(FCH):
                    nc.tensor.matmul(psy, wos[fb][:, j, :], obig[:, j, :],
                                     start=(j == 0), stop=(j == FCH - 1))
                ysb = yout.tile([128, LQ], F32, tag="y")
                nc.vector.tensor_scalar_add(ysb, psy, bo_sb[:, fb:fb + 1])
                nc.sync.dma_start(yT.ap()[ts(fb, 128), :], ysb)

    nc.compile()
    return nc


def _tables():
    """RoPE tables in halves-permuted basis: rows i (even-half) hold +sin,
    rows 32+i (odd-half) hold -sin (for the tmp-then-swap formulation)."""
    inv_freq = 1.0 / (10000.0 ** (np.arange(0, D, 2, dtype=np.float64) / D))
    ang = np.outer(inv_freq, np.arange(L, dtype=np.float64))  # [32, L]
    cos = np.cos(ang).astype(np.float32)
    sin = np.sin(ang).astype(np.float32)
    cos64 = np.concatenate([cos, cos], axis=0)                # [64, L]
    sin_sgn = np.concatenate([sin, -sin], axis=0)             # [64, L]
    return cos64, sin_sgn


def _prep_weights(Wq, bq, Wk, bk, Wv, bv, Wo, bo):
    perm = np.concatenate([np.arange(0, D, 2), np.arange(1, D, 2)])
    WqP = np.asarray(Wq, dtype=np.float32)[:, :, perm].reshape(F, H * D)
    bqP = np.asarray(bq, dtype=np.float32)[:, perm].reshape(H * D)
    WkP = np.asarray(Wk, dtype=np.float32)[:, perm]
    bkP = np.asarray(bk, dtype=np.float32)[perm]
    Wkv = np.concatenate([WkP, np.asarray(Wv, dtype=np.float32)], axis=1)
    bkv = np.concatenate([bkP, np.asarray(bv, dtype=np.float32)])
    WoR = np.asarray(Wo, dtype=np.float32).reshape(H * D, F)
    bo_ = np.asarray(bo, dtype=np.float32)

    wq_pret = np.ascontiguousarray(
        WqP.reshape(FCH, 128, FCH, 128).transpose(2, 1, 0, 3)).astype(
            ml_dtypes.bfloat16)
    wkv_pret = np.ascontiguousarray(
        Wkv.reshape(FCH, 128, 128).transpose(1, 0, 2)).astype(
            ml_dtypes.bfloat16)
    wo_pret = np.ascontiguousarray(
        WoR.reshape(FCH, 128, FCH, 128).transpose(2, 1, 0, 3)).astype(
            ml_dtypes.bfloat16)
    bqbo = np.ascontiguousarray(np.concatenate(
        [bqP.reshape(FCH, 128).T, bo_.reshape(FCH, 128).T], axis=1))
    return wq_pret, wkv_pret, wo_pret, bqbo, bkv


def kernel(inputs_q, inputs_kv, mask, Wq, bq, Wk, bk, Wv, bv, Wo, bo):
    if "nc" not in _CACHED:
        _CACHED["nc"] = build_nc()
    nc = _CACHED["nc"]

    wq_pret, wkv_pret, wo_pret, bqbo, bkv = _prep_weights(
        Wq, bq, Wk, bk, Wv, bv, Wo, bo)

    cos64, sin_sgn = _tables()
    scale = 1.0 / np.sqrt(np.float32(D))
    cksk = np.ascontiguousarray(
        np.concatenate([cos64, sin_sgn], axis=1))      # [64, 2*L] (L=LK)
    cosq_full = np.tile(cos64 * scale, (2, 1))         # [128, L]
    sinq_full = np.tile(sin_sgn * scale, (2, 1))

    xq = np.asarray(inputs_q, dtype=np.float32)
    xkv = np.asarray(inputs_kv, dtype=np.float32)
    mk = np.asarray(mask)

    in_maps = []
    for core in range(NCORES):
        b = core // 4
        qs = (core % 4) * LQ
        xq_t = np.ascontiguousarray(
            xq[b, qs:qs + LQ, :].T.reshape(FCH, 128, LQ)
            .transpose(1, 0, 2)).astype(ml_dtypes.bfloat16)
        xkv_t = np.ascontiguousarray(
            xkv[b].T.reshape(FCH, 128, NL, LQ).transpose(2, 1, 0, 3)).astype(
                ml_dtypes.bfloat16)
        mask_1 = (mk[b, 0, qs:qs + LQ, :].T.reshape(KCH, 128, LQ)
                  .transpose(1, 0, 2).astype(np.float16))    # [p, c, lq]
        mask_t = np.ascontiguousarray(
            np.broadcast_to(mask_1[:, :, None, :], (128, KCH, 2, LQ)))
        in_maps.append({
            "xq_t": xq_t,
            "xkv_t": xkv_t,
            "mask_t": mask_t,
            "wq": wq_pret,
            "wkv": wkv_pret,
            "wo": wo_pret,
            "bqbo": bqbo,
            "bkv": bkv,
            "cosq": np.ascontiguousarray(
                cosq_full[:, qs:qs + LQ]).astype(ml_dtypes.bfloat16),
            "sinq": np.ascontiguousarray(
                sinq_full[:, qs:qs + LQ]).astype(ml_dtypes.bfloat16),
            "cksk": cksk.astype(ml_dtypes.bfloat16),
        })

    res = bass_utils.run_bass_kernel_spmd(nc, in_maps,
                                          core_ids=list(range(NCORES)))
    _CACHED["last_results"] = res
    _CACHED["last_maps"] = in_maps

    out = np.empty((B, L, F), dtype=np.float32)
    for core in range(NCORES):
        b = core // 4
        qs = (core % 4) * LQ
        out[b, qs:qs + LQ, :] = res.results[core]["yT"].T
    return out
